# revision 6
# baseline (speedup 1.0000x reference)
"""DAG-SCM Trainium2 kernel.

Computes the reference nn_DAGSCM model: a 128-node topological scan
(x_i = relu(w.x_parents + b) + sigma_i * z_i) over n_samples, with the
per-node noise scale sigma_i calibrated from a tiny pilot pass
(0.1 * IQR, computed on host - it is a [128, 256] problem).

Strategy (memory-bound target):
  - Data-parallel over 8 NeuronCores on the sample axis.
  - Per core, samples live as [128 partitions x F free] tiles; each DAG
    node is one free-dim slice. The DAG structure and all per-node
    scalars (w0, w1, b, sigma) are baked into the traced Bass program
    as immediates / AP offsets at runtime (the kernel is JIT-traced
    with the actual inputs in hand).
  - Per non-root node (3 vector-engine-class ops + 1 gpsimd op):
        u  = w1 * p1 + b        (ScalarE activation, Identity)
        s  = w0 * p0 + u        (DVE scalar_tensor_tensor)
        zs = sigma * z          (GPSIMD tensor_scalar)
        v  = max(s, 0) + zs     (DVE scalar_tensor_tensor - fused relu+noise)
    Nodes in the output set write v directly into an interleaved output
    tile [p, f*64 + j] so the final DMA to the [n_samples, 64] output is
    fully contiguous per partition.
  - Only ancestors of the chosen output nodes are computed; only their
    z_noise rows are loaded (100 of 128 rows for this DAG).
"""

import os
import numpy as np

N_CORES = 8
P = 128  # SBUF partitions
CAL_FRAC = 0.1


def _host_pilot_sigma(W_eff, b, parents, is_root, root_pilot):
    """Noiseless pilot scan + per-node sigma = CAL_FRAC * IQR (host, f32)."""
    n_nodes = len(parents)
    n = root_pilot.shape[1]
    vals = np.zeros((n_nodes, n), np.float32)
    for i in range(n_nodes):
        if is_root[i]:
            v = root_pilot[i].astype(np.float32)
        else:
            h = np.zeros(n, np.float32)
            for p, w in parents[i]:
                h = h + np.float32(w) * vals[p]
            v = np.maximum(h + np.float32(b[i]), np.float32(0.0))
        v = np.where(np.isfinite(v), v, np.float32(0.0))
        vals[i] = v
    q75 = np.quantile(vals.astype(np.float64), 0.75, axis=1)
    q25 = np.quantile(vals.astype(np.float64), 0.25, axis=1)
    sigma = CAL_FRAC * np.maximum(q75 - q25, 1e-6)
    return sigma.astype(np.float32)


def _dag_structure(W, b, par_idx, par_mask, is_root, chosen):
    n_nodes = W.shape[0]
    W_eff = (np.asarray(W, np.float32) * np.asarray(par_mask, np.float32))
    parents = []
    for i in range(n_nodes):
        ps = [
            (int(par_idx[i, j]), float(W_eff[i, j]))
            for j in range(par_idx.shape[1])
            if par_mask[i, j] > 0
        ]
        parents.append(ps)
    # needed = chosen + all ancestors
    needed = set(int(c) for c in chosen)
    for i in range(n_nodes - 1, -1, -1):
        if i in needed and not is_root[i]:
            for p, _ in parents[i]:
                needed.add(p)
    return W_eff, parents, needed


def _build_program(NLOC, parents, is_root, chosen, needed, b, sigma, n_nodes,
                   op3_pool_mod=0, gz=10):
    """Trace the per-core Bass/Tile program. Returns (nc, z_rows, root_rows)."""
    from concourse import bacc
    import concourse.mybir as mybir
    from concourse.tile import TileContext

    F = NLOC // P
    assert NLOC % P == 0

    f32 = mybir.dt.float32
    AF = mybir.ActivationFunctionType
    OP = mybir.AluOpType

    # columns of the output each node must write (normally 0 or 1)
    node_cols = {}
    for j, c in enumerate(int(c) for c in chosen):
        node_cols.setdefault(c, []).append(j)
    n_out = len(chosen)

    topo = [i for i in range(n_nodes) if i in needed]  # index order == topo order
    z_rows = [i for i in topo if not is_root[i]]
    z_row_of = {node: r for r, node in enumerate(z_rows)}
    root_rows = [i for i in topo if is_root[i]]
    root_row_of = {node: r for r, node in enumerate(root_rows)}
    # nodes that need a contiguous vals slice: non-chosen needed nodes,
    # plus all roots (DMA target must be contiguous)
    vals_nodes = [i for i in topo if i not in node_cols or is_root[i]]
    n_z = len(z_rows)

    GZ = gz  # z rows per DMA group (~1.3 MB per dma_start at F=256)

    nc = bacc.Bacc(None, target_bir_lowering=False)
    z_in = nc.dram_tensor("zin", [max(n_z, 1), NLOC], f32, kind="ExternalInput")
    root_in = nc.dram_tensor("rootin", [max(len(root_rows), 1), NLOC], f32,
                             kind="ExternalInput")
    out_d = nc.dram_tensor("out", [NLOC, n_out], f32, kind="ExternalOutput")

    with TileContext(nc) as tc:
        with tc.tile_pool(name="vals", bufs=1) as vpool, \
             tc.tile_pool(name="zpool", bufs=3) as zpool, \
             tc.tile_pool(name="tmp", bufs=8) as tpool, \
             tc.tile_pool(name="outp", bufs=1) as opool:

            out_t = opool.tile([P, n_out * F], f32, tag="out", name="out_t")
            out_cols = out_t[:].rearrange("p (f j) -> p j f", j=n_out)

            vtile = {}
            for i in vals_nodes:
                vtile[i] = vpool.tile([P, F], f32, tag=f"v{i}", name=f"vt{i}")

            def col_ap(j):
                return out_cols[:, j]

            def src_ap(node):
                if node in vtile:
                    return vtile[node][:]
                return col_ap(node_cols[node][0])

            def dst_aps(node):
                if node in node_cols and not is_root[node]:
                    return [col_ap(j) for j in node_cols[node]]
                return [vtile[node][:]]

            # root rows: DMA straight into their vals slices
            for r in root_rows:
                nc.sync.dma_start(
                    out=vtile[r][:],
                    in_=root_in[root_row_of[r]:root_row_of[r] + 1, :]
                        .rearrange("o (p f) -> (o p) f", p=P),
                )
                for j in node_cols.get(r, []):
                    nc.vector.tensor_copy(out=col_ap(j), in_=vtile[r][:])

            # z row groups (node-order); DMA traced at group boundaries
            z_group_tiles = {}

            def ensure_z_group(g):
                if g in z_group_tiles:
                    return z_group_tiles[g]
                r0 = g * GZ
                r1 = min(r0 + GZ, n_z)
                zt = zpool.tile([P, (r1 - r0) * F], f32, tag="zg", name=f"zg{g}")
                nc.sync.dma_start(
                    out=zt[:].rearrange("p (r f) -> p r f", r=r1 - r0),
                    in_=z_in[r0:r1, :].rearrange("r (p f) -> p r f", p=P),
                )
                z_group_tiles[g] = zt
                return zt

            def z_ap(node):
                r = z_row_of[node]
                g, k = divmod(r, GZ)
                zt = ensure_z_group(g)
                return zt[:, k * F:(k + 1) * F]

            for i in topo:
                if is_root[i]:
                    continue
                ps = parents[i]
                bi = float(b[i])
                si = float(sigma[i])
                dsts = dst_aps(i)
                if len(ps) == 0:
                    # v = relu(b) + sigma*z in one op
                    c = max(bi, 0.0)
                    nc.vector.tensor_scalar(
                        out=dsts[0], in0=z_ap(i),
                        scalar1=si, scalar2=c, op0=OP.mult, op1=OP.add)
                else:
                    if len(ps) >= 2:
                        u_t = tpool.tile([P, F], f32, tag="u", name=f"u{i}")
                        nc.scalar.activation(
                            u_t[:], src_ap(ps[1][0]), AF.Identity,
                            bias=bi, scale=ps[1][1])
                        s_t = tpool.tile([P, F], f32, tag="s", name=f"s{i}")
                        nc.vector.scalar_tensor_tensor(
                            out=s_t[:], in0=src_ap(ps[0][0]), scalar=ps[0][1],
                            in1=u_t[:], op0=OP.mult, op1=OP.add)
                    else:
                        s_t = tpool.tile([P, F], f32, tag="s", name=f"s{i}")
                        nc.vector.tensor_scalar(
                            out=s_t[:], in0=src_ap(ps[0][0]),
                            scalar1=ps[0][1], scalar2=bi,
                            op0=OP.mult, op1=OP.add)
                    zs_t = tpool.tile([P, F], f32, tag="zs", name=f"zs{i}")
                    nc.gpsimd.tensor_scalar(
                        out=zs_t[:], in0=z_ap(i),
                        scalar1=si, scalar2=None, op0=OP.mult)
                    # v = max(s, 0) + zs  (fused relu + noise add)
                    op3_eng = nc.gpsimd if (
                        op3_pool_mod and z_row_of[i] % op3_pool_mod == 0
                    ) else nc.vector
                    op3_eng.scalar_tensor_tensor(
                        out=dsts[0], in0=s_t[:], scalar=0.0, in1=zs_t[:],
                        op0=OP.max, op1=OP.add)
                for extra in dsts[1:]:
                    nc.vector.tensor_copy(out=extra, in_=dsts[0])

            # output DMA: all 128 partitions per transfer (full SBUF port
            # parallelism), split along the free dim across several
            # dma_starts
            out_ap = out_d[:, :].rearrange("(p f) j -> p (f j)", p=P)
            FSPLIT = (F + 7) // 8
            for f0 in range(0, F, FSPLIT):
                f1 = min(f0 + FSPLIT, F)
                nc.sync.dma_start(
                    out=out_ap[:, f0 * n_out:f1 * n_out],
                    in_=out_t[:, f0 * n_out:f1 * n_out])

    nc.finalize()
    return nc, z_rows, root_rows


_CACHE = {}
_LAST_NC = None
_LAST_IN_MAPS = None


def _get_program(key, *args):
    if key not in _CACHE:
        _CACHE[key] = _build_program(*args)
    return _CACHE[key]


def run(n_samples, W, b, root_pilot, root_main, z_noise, par_mask, par_idx,
        is_root, chosen, trace=False, n_cores=N_CORES, op3_pool_mod=0, gz=10):
    W = np.asarray(W, np.float32)
    b = np.asarray(b, np.float32)
    root_pilot = np.asarray(root_pilot, np.float32)
    root_main = np.asarray(root_main, np.float32)
    z_noise = np.asarray(z_noise, np.float32)
    par_mask = np.asarray(par_mask, np.float32)
    par_idx = np.asarray(par_idx, np.int32)
    is_root = np.asarray(is_root, bool)
    chosen = np.asarray(chosen, np.int32)

    n_nodes = W.shape[0]
    NS = root_main.shape[1]
    assert NS % (n_cores * P) == 0
    NLOC = NS // n_cores

    W_eff, parents, needed = _dag_structure(W, b, par_idx, par_mask, is_root,
                                            chosen)
    sigma = _host_pilot_sigma(W_eff, b, parents, is_root, root_pilot)

    key = (NLOC, n_nodes, tuple(chosen.tolist()), par_idx.tobytes(),
           par_mask.tobytes(), W_eff.tobytes(), b.tobytes(), sigma.tobytes(),
           is_root.tobytes(), op3_pool_mod, gz)
    nc, z_rows, root_rows = _get_program(
        key, NLOC, parents, is_root, chosen, needed, b, sigma, n_nodes,
        op3_pool_mod, gz)

    z_packed = np.ascontiguousarray(z_noise[z_rows]) if z_rows else \
        np.zeros((1, NS), np.float32)
    root_packed = np.ascontiguousarray(root_main[root_rows]) if root_rows \
        else np.zeros((1, NS), np.float32)

    in_maps = []
    for c in range(n_cores):
        s0, s1 = c * NLOC, (c + 1) * NLOC
        in_maps.append({
            "zin": np.ascontiguousarray(z_packed[:, s0:s1]),
            "rootin": np.ascontiguousarray(root_packed[:, s0:s1]),
        })

    from concourse.bass_utils import run_bass_kernel_spmd
    global _LAST_NC, _LAST_IN_MAPS
    _LAST_NC, _LAST_IN_MAPS = nc, in_maps
    res = run_bass_kernel_spmd(nc, in_maps, core_ids=list(range(n_cores)),
                               trace=trace)
    out = np.concatenate([np.asarray(r["out"]) for r in res.results], axis=0)
    return out.astype(np.float32, copy=False), res


def kernel(**inputs):
    out, _ = run(**inputs)
    return out
